# revision 1
# baseline (speedup 1.0000x reference)
"""Single-head attention (B=4, S=2048, E=1024) on 8 TRN2 NeuronCores.

Sharding: core c -> (batch b = c//2, sequence-half h = c%2).

Core c -> (batch b = c//2, half h = c%2). Each core computes K^T and V
only for its OWN 1024-column half of the sequence, then the pair
all-gathers both (through DRAM bounce buffers) so each core assembles
the full K^T [1024f, 2048k] and V [2048k, 1024f] in ABSOLUTE key order
(gather block hh is rank hh's half -> no per-core indexing anywhere;
the SPMD program is identical across cores).

Saves 256 of 1184 matmuls per core vs the duplicate-K/V version and
cuts input DMA from 14MB to 8MB (x ships as just the core's own half).
"""

import numpy as np
import ml_dtypes

import concourse.bass as bass
import concourse.tile as tile
from concourse import bacc, mybir
from concourse.bass_utils import run_bass_kernel_spmd

B, S, E = 4, 2048, 1024
N_CORES = 8
SQ = S // 2
P = 128
NT = 512
ET = E // P        # 8
KT = S // P        # 16
KTH = SQ // P      # 8 own-half k tiles
FP32 = mybir.dt.float32
BF16 = mybir.dt.bfloat16
SCALE = 1.0 / np.sqrt(E).astype(np.float32)
SHIFT = -4.0
PAIRS = [[0, 1], [2, 3], [4, 5], [6, 7]]


def build_kernel(ctx, tc, io):
    nc = tc.nc
    xo, wqT, wkT, wvT, bq, bk, bv, outT = (
        io["xo"], io["wqT"], io["wkT"], io["wvT"],
        io["bq"], io["bk"], io["bv"], io["outT"],
    )

    singles = ctx.enter_context(tc.tile_pool(name="singles", bufs=1))
    results = ctx.enter_context(tc.tile_pool(name="results", bufs=1))
    # kh/vh halves are dead once bounced out to DRAM; p_sb reuses the slot.
    xp_pool = ctx.enter_context(tc.tile_pool(name="xp", bufs=1))
    outp = ctx.enter_context(tc.tile_pool(name="outp", bufs=3))
    dram = ctx.enter_context(tc.tile_pool(name="dram", bufs=1, space="DRAM"))
    ps_main = ctx.enter_context(tc.tile_pool(name="ps_main", bufs=4, space="PSUM"))
    ps_sums = ctx.enter_context(tc.tile_pool(name="ps_sums", bufs=2, space="PSUM"))
    ps_out = ctx.enter_context(tc.tile_pool(name="ps_out", bufs=2, space="PSUM"))

    # ---- ScalarE LUT warm-up: force the Identity/Exp ACT_TABLE_LOADs to
    # happen before the input DMA stream, not queued behind it (a late
    # table load stalls every ACTIVATE -> PSUM recycle -> TensorE).
    warm = singles.tile([1, 4], FP32)
    warmb = singles.tile([1, 1], FP32)
    nc.vector.memset(warm, 0.0)
    nc.vector.memset(warmb, 0.0)
    nc.scalar.activation(out=warm[:, 0:2], in_=warm[:, 0:2],
                         func=mybir.ActivationFunctionType.Identity,
                         bias=warmb, scale=1.0)
    nc.scalar.activation(out=warm[:, 2:4], in_=warm[:, 2:4],
                         func=mybir.ActivationFunctionType.Exp,
                         bias=warmb, scale=1.0)

    # ---- input staging, first-use order. Tiny bias DMAs go FIRST (they
    # gate the first ACTIVATEs; queued behind the slabs they land ~35us in).
    bq_sb = singles.tile([P, ET], FP32)
    bk_sb = singles.tile([P, ET], FP32)
    bv_bc = singles.tile([P, E], FP32)
    nc.gpsimd.dma_start(out=bk_sb, in_=bk.rearrange("(t p) -> p t", p=P))
    nc.gpsimd.dma_start(out=bq_sb, in_=bq.rearrange("(t p) -> p t", p=P))
    nc.gpsimd.dma_start(out=bv_bc, in_=bv.partition_broadcast(P))

    # Slab DMAs alternate between sync and vector queues: issue costs
    # ~0.6us per dma_start per engine, so single-engine issue of 16 slabs
    # would serialize ~10us against an 11us transfer floor.
    wk_sb = singles.tile([P, ET, E], BF16)
    wv_sb = singles.tile([P, ET, E], BF16)
    wq_sb = singles.tile([P, ET, E], BF16)
    xo_sb = singles.tile([P, ET, SQ], BF16)
    for t in range(ET):
        r = slice(t * P, (t + 1) * P)
        nc.sync.dma_start(out=wk_sb[:, t, :], in_=wkT[r, :])
        nc.sync.dma_start(out=xo_sb[:, t, :], in_=xo[r, :])
    for t in range(ET):
        r = slice(t * P, (t + 1) * P)
        nc.sync.dma_start(out=wv_sb[:, t, :], in_=wvT[r, :])
    for t in range(ET):
        r = slice(t * P, (t + 1) * P)
        nc.sync.dma_start(out=wq_sb[:, t, :], in_=wqT[r, :])

    ones_sb = singles.tile([P, 1], BF16)
    nc.vector.memset(ones_sb, 1.0)
    shift_sb = singles.tile([P, 1], FP32)
    nc.vector.memset(shift_sb, SHIFT)

    qT_sb = results.tile([P, ET, SQ], BF16)
    kT_sb = results.tile([P, ET, S], BF16)
    v_sb = results.tile([P, KT, E], BF16)
    scr = xp_pool.tile([P, KT, SQ], BF16, tag="xp")  # kh: [:, 0:8, :], vh: [:, 8:16, :]
    kh_sb = scr[:, 0:ET, :]
    vh_sb = scr[:, ET:KT, :]

    # K gather is split into two k'-halves so the collective firmware
    # latency pipelines against the remaining projections; V is one gather
    # (only needed by PV, much later).
    bounce_k = [dram.tile([SQ, NT], BF16, name=f"bounce_k{i}", tag=f"bk{i}")
                for i in range(2)]
    gath_k = [dram.tile([S, NT], BF16, name=f"gath_k{i}", tag=f"gk{i}")
              for i in range(2)]
    bounce_v = dram.tile([SQ, E], BF16)
    gath_v = dram.tile([S, E], BF16)

    ident = mybir.ActivationFunctionType.Identity

    # ---- K^T own half [f, k'], one k'-half at a time: compute -> bounce
    # -> gather -> back-DMA (absolute k order in kT_sb)
    for ks in range(SQ // NT):
        kr = slice(ks * NT, (ks + 1) * NT)
        for ft in range(ET):
            fr = slice(ft * P, (ft + 1) * P)
            ps = ps_main.tile([P, NT], FP32)
            for et in range(ET):
                nc.tensor.matmul(ps, lhsT=wk_sb[:, et, fr], rhs=xo_sb[:, et, kr],
                                 start=(et == 0), stop=(et == ET - 1))
            nc.scalar.activation(out=kh_sb[:, ft, kr], in_=ps, func=ident,
                                 bias=bk_sb[:, ft:ft + 1], scale=1.0)
        for t in range(ET):
            nc.sync.dma_start(out=bounce_k[ks][t * P:(t + 1) * P, :],
                              in_=kh_sb[:, t, kr])
        nc.gpsimd.collective_compute(
            "AllGather", mybir.AluOpType.bypass, replica_groups=PAIRS,
            ins=[bounce_k[ks][:, :]], outs=[gath_k[ks][:, :]])
        for hh in range(2):
            for t in range(ET):
                nc.sync.dma_start(
                    out=kT_sb[:, t, hh * SQ + ks * NT: hh * SQ + (ks + 1) * NT],
                    in_=gath_k[ks][hh * SQ + t * P: hh * SQ + (t + 1) * P, :])

    # ---- V own half  [k', f] -> bounce -> gather -> back
    for kt in range(KTH):
        kr = slice(kt * P, (kt + 1) * P)
        for fs in range(E // NT):
            fr = slice(fs * NT, (fs + 1) * NT)
            ps = ps_main.tile([P, NT], FP32)
            for et in range(ET):
                nc.tensor.matmul(ps, lhsT=xo_sb[:, et, kr], rhs=wv_sb[:, et, fr],
                                 start=(et == 0), stop=(et == ET - 1))
            nc.vector.tensor_add(vh_sb[:, kt, fr], ps, bv_bc[:, fr])
        nc.sync.dma_start(out=bounce_v[kt * P:(kt + 1) * P, :], in_=vh_sb[:, kt, :])
    nc.gpsimd.collective_compute(
        "AllGather", mybir.AluOpType.bypass, replica_groups=PAIRS,
        ins=[bounce_v[:, :]], outs=[gath_v[:, :]])
    for kt in range(KT):
        nc.sync.dma_start(out=v_sb[:, kt, :], in_=gath_v[kt * P:(kt + 1) * P, :])

    # ---- Q^T = Wq x_own + bq   [f, q]  (overlaps the gathers)
    # qs-outer: all of qs=0 lands first so scores(qs=0) isn't gated on the
    # last ACT of the whole Q pass.
    for qs in range(SQ // NT):
        qr = slice(qs * NT, (qs + 1) * NT)
        for ft in range(ET):
            fr = slice(ft * P, (ft + 1) * P)
            ps = ps_main.tile([P, NT], FP32)
            for et in range(ET):
                nc.tensor.matmul(ps, lhsT=wq_sb[:, et, fr], rhs=xo_sb[:, et, qr],
                                 start=(et == 0), stop=(et == ET - 1))
            nc.scalar.activation(out=qT_sb[:, ft, qr], in_=ps, func=ident,
                                 bias=bq_sb[:, ft:ft + 1], scale=1.0)

    # ---- scores^T and P = exp(S^T * scale + shift)   [k, q]
    # qs-outer: finish all kt for one q-slice, then its denominators,
    # so the sums/PV phases aren't gated on the very last exp of both
    # q-slices.
    p_sb = xp_pool.tile([P, KT, SQ], BF16, tag="xp")
    recip_sb = singles.tile([1, SQ], FP32)
    for qs in range(SQ // NT):
        qr = slice(qs * NT, (qs + 1) * NT)
        for kt in range(KT):
            kr = slice(kt * P, (kt + 1) * P)
            ps = ps_main.tile([P, NT], FP32)
            for et in range(ET):
                nc.tensor.matmul(ps, lhsT=kT_sb[:, et, kr], rhs=qT_sb[:, et, qr],
                                 start=(et == 0), stop=(et == ET - 1))
            nc.scalar.activation(out=p_sb[:, kt, qr], in_=ps,
                                 func=mybir.ActivationFunctionType.Exp,
                                 bias=shift_sb[:, 0:1], scale=float(SCALE))
        ps_d = ps_sums.tile([1, NT], FP32)
        for kt in range(KT):
            nc.tensor.matmul(ps_d, lhsT=ones_sb, rhs=p_sb[:, kt, qr],
                             start=(kt == 0), stop=(kt == KT - 1))
        nc.vector.reciprocal(out=recip_sb[:, qr], in_=ps_d)
    recip_dram = dram.tile([1, SQ], FP32)
    nc.sync.dma_start(out=recip_dram, in_=recip_sb)
    recip_bc = singles.tile([P, SQ], FP32)
    nc.sync.dma_start(out=recip_bc, in_=recip_dram[0, :].partition_broadcast(P))

    # ---- O^T = V^T P, normalize, out
    for ft in range(ET):
        fr = slice(ft * P, (ft + 1) * P)
        for qs in range(SQ // NT):
            qr = slice(qs * NT, (qs + 1) * NT)
            ps = ps_out.tile([P, NT], FP32)
            for kt in range(KT):
                nc.tensor.matmul(ps, lhsT=v_sb[:, kt, fr], rhs=p_sb[:, kt, qr],
                                 start=(kt == 0), stop=(kt == KT - 1))
            ot = outp.tile([P, NT], FP32)
            nc.vector.tensor_mul(ot, ps, recip_bc[:, qr])
            nc.sync.dma_start(out=outT[fr, qr], in_=ot)


def build_program():
    nc = bacc.Bacc("TRN2", target_bir_lowering=False, debug=False,
                   num_devices=N_CORES)
    io = {
        "xo": nc.dram_tensor("xo", [E, SQ], BF16, kind="ExternalInput").ap(),
        "wqT": nc.dram_tensor("wqT", [E, E], BF16, kind="ExternalInput").ap(),
        "wkT": nc.dram_tensor("wkT", [E, E], BF16, kind="ExternalInput").ap(),
        "wvT": nc.dram_tensor("wvT", [E, E], BF16, kind="ExternalInput").ap(),
        "bq": nc.dram_tensor("bq", [E], FP32, kind="ExternalInput").ap(),
        "bk": nc.dram_tensor("bk", [E], FP32, kind="ExternalInput").ap(),
        "bv": nc.dram_tensor("bv", [E], FP32, kind="ExternalInput").ap(),
        "outT": nc.dram_tensor("outT", [E, SQ], FP32, kind="ExternalOutput").ap(),
    }
    from contextlib import ExitStack
    with tile.TileContext(nc) as tc:
        with ExitStack() as ctx:
            build_kernel(ctx, tc, io)
    nc.compile()
    return nc


def make_in_maps(x, wq_w, wq_b, wk_w, wk_b, wv_w, wv_b):
    bf = ml_dtypes.bfloat16
    xT_all = np.ascontiguousarray(np.transpose(np.asarray(x, np.float32),
                                               (0, 2, 1))).astype(bf)
    wqT = np.ascontiguousarray(np.asarray(wq_w, np.float32).T).astype(bf)
    wkT = np.ascontiguousarray(np.asarray(wk_w, np.float32).T).astype(bf)
    wvT = np.ascontiguousarray(np.asarray(wv_w, np.float32).T).astype(bf)
    bq = np.asarray(wq_b, np.float32)
    bk = np.asarray(wk_b, np.float32)
    bv = np.asarray(wv_b, np.float32)
    in_maps = []
    for c in range(N_CORES):
        b, h = divmod(c, 2)
        in_maps.append({
            "xo": np.ascontiguousarray(xT_all[b][:, h * SQ:(h + 1) * SQ]),
            "wqT": wqT, "wkT": wkT, "wvT": wvT,
            "bq": bq, "bk": bk, "bv": bv,
        })
    return in_maps


def assemble_out(results):
    out = np.empty((B, S, E), np.float32)
    for c in range(N_CORES):
        b, h = divmod(c, 2)
        out[b, h * SQ:(h + 1) * SQ, :] = results[c]["outT"].T
    return out


_NC_CACHE = None


def kernel(x, wq_w, wq_b, wk_w, wk_b, wv_w, wv_b):
    global _NC_CACHE
    if _NC_CACHE is None:
        _NC_CACHE = build_program()
    in_maps = make_in_maps(x, wq_w, wq_b, wk_w, wk_b, wv_w, wv_b)
    try:
        res = run_bass_kernel_spmd(_NC_CACHE, in_maps, list(range(N_CORES)))
    except Exception:
        # transient axon/device hiccups happen; one retry
        res = run_bass_kernel_spmd(_NC_CACHE, in_maps, list(range(N_CORES)))
    return assemble_out(res.results)



# revision 2
# speedup vs baseline: 1.0150x; 1.0150x over previous
"""Single-head attention (B=4, S=2048, E=1024) on 8 TRN2 NeuronCores.

Sharding: core c -> (batch b = c//2, sequence-half h = c%2).

v5: CC warm-up dummy, split kh/vh scratch slots, earlier first slab. v3 notes: stream runs at the 216ns/MM roofline, so v3 removes
the remaining stalls found in the v2 trace:
  - wq/wv ship as slab DMAs QUEUED BEHIND wk/xo on the same channels:
    v2's batched wq/wv transfers ran concurrently and stole HBM
    bandwidth from the first-phase slabs (3.7us of et-outer stalls).
  - K gather-backs moved to the gpsimd queue: in v2 they sat AHEAD of
    bounce_v in the sync FIFO while waiting on the K collectives, and
    the first scores exp (WAR on the shared kh/vh scratch) stalled
    2.9us behind them.
  - softmax denominators: DVE running-sum over the 16 exp tiles (off
    the PE path) + ONE ones[128,128] matmul per q-slice that both
    cross-partition-reduces and broadcasts -> saves 30 of 32
    denominator matmuls (6.5us) AND kills the reciprocal DRAM
    round-trip (reciprocal reads the broadcast PSUM directly).
  - PV is qs-outer so qs=1's reciprocal has 27us of slack.
"""

import numpy as np
import ml_dtypes

import concourse.bass as bass
import concourse.tile as tile
from concourse import bacc, mybir
from concourse.bass_utils import run_bass_kernel_spmd

B, S, E = 4, 2048, 1024
N_CORES = 8
SQ = S // 2
P = 128
NT = 512
ET = E // P        # 8
KT = S // P        # 16
KTH = SQ // P      # 8 own-half k tiles
FP32 = mybir.dt.float32
FP32R = mybir.dt.float32r
BF16 = mybir.dt.bfloat16
SCALE = 1.0 / np.sqrt(E).astype(np.float32)
SHIFT = -4.0
PAIRS = [[0, 1], [2, 3], [4, 5], [6, 7]]


def build_kernel(ctx, tc, io):
    nc = tc.nc
    xo, wqT, wkT, wvT, bq, bk, bv, outT = (
        io["xo"], io["wqT"], io["wkT"], io["wvT"],
        io["bq"], io["bk"], io["bv"], io["outT"],
    )

    singles = ctx.enter_context(tc.tile_pool(name="singles", bufs=1))
    results = ctx.enter_context(tc.tile_pool(name="results", bufs=1))
    xp_pool = ctx.enter_context(tc.tile_pool(name="xp", bufs=1))
    outp = ctx.enter_context(tc.tile_pool(name="outp", bufs=3))
    dram = ctx.enter_context(tc.tile_pool(name="dram", bufs=1, space="DRAM"))
    ps_a = ctx.enter_context(tc.tile_pool(name="ps_a", bufs=4, space="PSUM"))
    ps_b = ctx.enter_context(tc.tile_pool(name="ps_b", bufs=2, space="PSUM"))
    ps_c = ctx.enter_context(tc.tile_pool(name="ps_c", bufs=2, space="PSUM"))

    # ---- ScalarE LUT warm-up: force Identity/Exp ACT_TABLE_LOADs before
    # the input DMA stream.
    warm = singles.tile([1, 4], FP32)
    warmb = singles.tile([1, 1], FP32)
    nc.vector.memset(warm, 0.0)
    nc.vector.memset(warmb, 0.0)
    nc.scalar.activation(out=warm[:, 0:2], in_=warm[:, 0:2],
                         func=mybir.ActivationFunctionType.Identity,
                         bias=warmb, scale=1.0)
    nc.scalar.activation(out=warm[:, 2:4], in_=warm[:, 2:4],
                         func=mybir.ActivationFunctionType.Exp,
                         bias=warmb, scale=1.0)

    # Tiny dummy collective fired at ~8us: absorbs the 11-18us CC
    # firmware start-delay so the real K0 gather starts promptly. Its
    # 16-byte input ships from the (already memset) warm tile as the
    # very first sync DMA.
    warm_cc_in = dram.tile([1, 1], FP32, name="warm_cc_in")
    warm_cc_out = dram.tile([2, 1], FP32, name="warm_cc_out")
    nc.sync.dma_start(out=warm_cc_in, in_=warmb[0:1, :])
    nc.gpsimd.collective_compute(
        "AllGather", mybir.AluOpType.bypass, replica_groups=PAIRS,
        ins=[warm_cc_in[:, :]], outs=[warm_cc_out[:, :]])

    # ---- input staging. Tiny bias DMAs first (they gate the first ACTs).
    bq_sb = singles.tile([P, ET], FP32)
    bk_sb = singles.tile([P, ET], FP32)
    bv_bc = singles.tile([P, E], FP32)
    nc.gpsimd.dma_start(out=bk_sb, in_=bk.rearrange("(t p) -> p t", p=P))
    nc.gpsimd.dma_start(out=bq_sb, in_=bq.rearrange("(t p) -> p t", p=P))
    nc.gpsimd.dma_start(out=bv_bc, in_=bv.partition_broadcast(P))

    # wk/xo slabs feed the et-outer first phase; wq/wv slabs go BEHIND
    # them on the same queues so their transfers can't steal HBM
    # bandwidth from the critical first 4MB.
    wk_sb = singles.tile([P, ET, E], BF16)
    wv_sb = singles.tile([P, ET, E], BF16)
    wq_sb = singles.tile([P, ET, E], BF16)
    xo_sb = singles.tile([P, ET, SQ], BF16)
    # The ks=0 et-outer phase only needs wk (2MB) + the FIRST COLUMN
    # HALF of xo (1MB): ship xo as column-halves so the critical set is
    # 3MB and slab delivery stays ahead of the warm PE (1.73us/step).
    # Everything else (xo 2nd half, wq, wv) trails on the scalar queue
    # FIFO; sync stays free so the K bounces transfer promptly.
    def chunk(q, dst_sb, src, t, n, cols):
        r = slice(t * P, (t + n) * P)
        q.dma_start(out=dst_sb[:, t:t + n, cols],
                    in_=src[r, cols].rearrange("(t p) f -> p t f", p=P))
    for (t, n) in [(0, 1), (1, 2), (3, 2), (5, 2), (7, 1)]:
        chunk(nc.sync, wk_sb, wkT, t, n, slice(None))
        chunk(nc.scalar, xo_sb, xo, t, n, slice(0, NT))
    nc.scalar.dma_start(out=xo_sb[:, :, NT:],
                        in_=xo[:, NT:].rearrange("(t p) f -> p t f", p=P))
    nc.scalar.dma_start(out=wq_sb, in_=wqT.rearrange("(t p) f -> p t f", p=P))
    nc.scalar.dma_start(out=wv_sb, in_=wvT.rearrange("(t p) f -> p t f", p=P))

    ones_b = singles.tile([P, P], BF16)
    nc.vector.memset(ones_b, 1.0)
    shift_sb = singles.tile([P, 1], FP32)
    nc.vector.memset(shift_sb, SHIFT)

    qT_sb = results.tile([P, ET, SQ], BF16)
    kT_sb = results.tile([P, ET, S], BF16)
    v_sb = results.tile([P, KT, E], BF16)
    # kh/vh scratch in two separate pool slots so the exp outputs that
    # recycle the kh slot don't WAR-wait on the (late) bounce_v read.
    kh_sb = xp_pool.tile([P, ET, SQ], BF16, tag="xpk")
    vh_sb = xp_pool.tile([P, ET, SQ], BF16, tag="xpv")

    bounce_k = [dram.tile([SQ, NT], BF16, name=f"bounce_k{i}", tag=f"bk{i}")
                for i in range(2)]
    gath_k = [dram.tile([S, NT], BF16, name=f"gath_k{i}", tag=f"gk{i}")
              for i in range(2)]
    bounce_v = dram.tile([SQ, E], BF16)
    gath_v = dram.tile([S, E], BF16)

    ident = mybir.ActivationFunctionType.Identity

    # ---- K^T own half [f, k'].
    # ks=0: contraction(et)-outer across all 8 PSUM banks -> first matmul
    # only needs slab 0 of wk/xo. ks=1: ft-outer (slabs all resident).
    ks0 = slice(0, NT)
    ps_k = ([ps_a.tile([P, NT], FP32, name=f"ps_ka{i}", tag="psa")
             for i in range(4)]
            + [ps_b.tile([P, NT], FP32, name=f"ps_kb{i}", tag="psb")
               for i in range(2)]
            + [ps_c.tile([P, NT], FP32, name=f"ps_kc{i}", tag="psc")
               for i in range(2)])
    for et in range(ET):
        for ft in range(ET):
            nc.tensor.matmul(ps_k[ft], lhsT=wk_sb[:, et, ft * P:(ft + 1) * P],
                             rhs=xo_sb[:, et, ks0],
                             start=(et == 0), stop=(et == ET - 1))
    for ft in range(ET):
        nc.scalar.activation(out=kh_sb[:, ft, ks0], in_=ps_k[ft], func=ident,
                             bias=bk_sb[:, ft:ft + 1], scale=1.0)
    nc.sync.dma_start(out=bounce_k[0].rearrange("(t p) n -> p t n", p=P),
                      in_=kh_sb[:, :, ks0])
    nc.gpsimd.collective_compute(
        "AllGather", mybir.AluOpType.bypass, replica_groups=PAIRS,
        ins=[bounce_k[0][:, :]], outs=[gath_k[0][:, :]])

    ks1 = slice(NT, SQ)
    for ft in range(ET):
        fr = slice(ft * P, (ft + 1) * P)
        ps = ps_a.tile([P, NT], FP32, tag="psa")
        for et in range(ET):
            nc.tensor.matmul(ps, lhsT=wk_sb[:, et, fr], rhs=xo_sb[:, et, ks1],
                             start=(et == 0), stop=(et == ET - 1))
        nc.scalar.activation(out=kh_sb[:, ft, ks1], in_=ps, func=ident,
                             bias=bk_sb[:, ft:ft + 1], scale=1.0)
    nc.sync.dma_start(out=bounce_k[1].rearrange("(t p) n -> p t n", p=P),
                      in_=kh_sb[:, :, ks1])
    nc.gpsimd.collective_compute(
        "AllGather", mybir.AluOpType.bypass, replica_groups=PAIRS,
        ins=[bounce_k[1][:, :]], outs=[gath_k[1][:, :]])

    # ---- V own half [k', f] -> bounce (sync) -> gather (gpsimd)
    for kt in range(KTH):
        kr = slice(kt * P, (kt + 1) * P)
        for fs in range(E // NT):
            fr = slice(fs * NT, (fs + 1) * NT)
            ps = ps_a.tile([P, NT], FP32, tag="psa")
            for et in range(ET):
                nc.tensor.matmul(ps, lhsT=xo_sb[:, et, kr], rhs=wv_sb[:, et, fr],
                                 start=(et == 0), stop=(et == ET - 1))
            nc.vector.tensor_add(vh_sb[:, kt, fr], ps, bv_bc[:, fr])
    nc.sync.dma_start(out=bounce_v.rearrange("(t p) f -> p t f", p=P),
                      in_=vh_sb[:, :, :])
    nc.gpsimd.collective_compute(
        "AllGather", mybir.AluOpType.bypass, replica_groups=PAIRS,
        ins=[bounce_v[:, :]], outs=[gath_v[:, :]])

    # K gather-backs on gpsimd, AFTER the V collective trigger in queue
    # order (they wait on the K collectives anyway; keeping them off the
    # sync queue unblocks bounce_v).
    for ks in range(2):
        for hh in range(2):
            lo = hh * SQ + ks * NT
            nc.gpsimd.dma_start(
                out=kT_sb[:, :, lo:lo + NT],
                in_=gath_k[ks][hh * SQ:(hh + 1) * SQ, :]
                .rearrange("(t p) n -> p t n", p=P))
    # V gather-back on sync (idle after bounce_v), in halves so the
    # first PV chains start as soon as kt 0-7 land.
    for vh2 in range(2):
        lo = vh2 * KTH
        nc.sync.dma_start(
            out=v_sb[:, lo:lo + KTH, :],
            in_=gath_v[lo * P:(lo + KTH) * P, :]
            .rearrange("(t p) f -> p t f", p=P))

    # ---- Q^T = Wq x_own + bq   [f, q]  (overlaps the gathers)
    for qs in range(SQ // NT):
        qr = slice(qs * NT, (qs + 1) * NT)
        for ft in range(ET):
            fr = slice(ft * P, (ft + 1) * P)
            ps = ps_a.tile([P, NT], FP32, tag="psa")
            for et in range(ET):
                nc.tensor.matmul(ps, lhsT=wq_sb[:, et, fr], rhs=xo_sb[:, et, qr],
                                 start=(et == 0), stop=(et == ET - 1))
            nc.scalar.activation(out=qT_sb[:, ft, qr], in_=ps, func=ident,
                                 bias=bq_sb[:, ft:ft + 1], scale=1.0)

    # ---- scores^T and P = exp(S^T * scale + shift)   [k, q]
    # Denominator: DVE running sum over the 16 exp tiles (acc, fp32r so
    # it can feed a matmul directly), then ONE ones[128,128] matmul that
    # cross-partition-reduces AND broadcasts the result to all
    # partitions; reciprocal reads that PSUM directly.  The denominator
    # matmul for qs is issued a few chains into the NEXT phase so the PE
    # never waits on the DVE accumulation.
    p_k = xp_pool.tile([P, ET, SQ], BF16, tag="xpk")
    p_v = xp_pool.tile([P, ET, SQ], BF16, tag="xpv")

    def p_tile(kt, qr):
        return p_k[:, kt, qr] if kt < ET else p_v[:, kt - ET, qr]
    acc = singles.tile([P, 2, NT], FP32, name="acc_den")
    acc_b = singles.tile([P, 2, NT], BF16, name="acc_den_b")
    recip_bc = singles.tile([P, SQ], FP32)
    ps_den = [None, None]

    # kt order visits the ks=0-gathered blocks first: the ks=1 gather-
    # backs land ~15us later (CC stream is serialized) and softmax is
    # k-order-invariant, so consume what's ready first.
    KT_ORDER = [0, 1, 2, 3, 8, 9, 10, 11, 4, 5, 6, 7, 12, 13, 14, 15]

    def scores_phase(qs):
        qr = slice(qs * NT, (qs + 1) * NT)
        for ki, kt in enumerate(KT_ORDER):
            kr = slice(kt * P, (kt + 1) * P)
            ps = ps_a.tile([P, NT], FP32, tag="psa")
            for et in range(ET):
                nc.tensor.matmul(ps, lhsT=kT_sb[:, et, kr], rhs=qT_sb[:, et, qr],
                                 start=(et == 0), stop=(et == ET - 1))
            nc.scalar.activation(out=p_tile(kt, qr), in_=ps,
                                 func=mybir.ActivationFunctionType.Exp,
                                 bias=shift_sb[:, 0:1], scale=float(SCALE))
            # DVE running sum (off the PE path)
            if ki == 1:
                nc.vector.tensor_add(acc[:, qs, :], p_tile(KT_ORDER[0], qr),
                                     p_tile(kt, qr))
            elif ki > 1:
                nc.vector.tensor_add(acc[:, qs, :], acc[:, qs, :],
                                     p_tile(kt, qr))
                if ki == KT - 1:
                    nc.vector.tensor_copy(out=acc_b[:, qs, :],
                                          in_=acc[:, qs, :])
            if qs == 1 and ki == 3:
                denominator_matmul(0)

    def denominator_matmul(qs):
        qr = slice(qs * NT, (qs + 1) * NT)
        ps_den[qs] = ps_b.tile([P, NT], FP32, tag="psb", name=f"ps_den{qs}")
        nc.tensor.matmul(ps_den[qs], lhsT=ones_b, rhs=acc_b[:, qs, :],
                         start=True, stop=True)
        nc.vector.reciprocal(out=recip_bc[:, qr], in_=ps_den[qs])

    scores_phase(0)
    scores_phase(1)

    # ---- O^T = V^T P, normalize, out (qs-outer: qs=1's reciprocal has
    # a whole qs=0 sweep of slack)
    for qs in range(SQ // NT):
        qr = slice(qs * NT, (qs + 1) * NT)
        for ft in range(ET):
            fr = slice(ft * P, (ft + 1) * P)
            ps = ps_c.tile([P, NT], FP32, tag="psc")
            for kt in range(KT):
                nc.tensor.matmul(ps, lhsT=v_sb[:, kt, fr], rhs=p_tile(kt, qr),
                                 start=(kt == 0), stop=(kt == KT - 1))
            if qs == 0 and ft == 0:
                denominator_matmul(1)
            ot = outp.tile([P, NT], FP32)
            nc.vector.tensor_mul(ot, ps, recip_bc[:, qr])
            nc.scalar.dma_start(out=outT[fr, qr], in_=ot)


def build_program():
    nc = bacc.Bacc("TRN2", target_bir_lowering=False, debug=False,
                   num_devices=N_CORES)
    io = {
        "xo": nc.dram_tensor("xo", [E, SQ], BF16, kind="ExternalInput").ap(),
        "wqT": nc.dram_tensor("wqT", [E, E], BF16, kind="ExternalInput").ap(),
        "wkT": nc.dram_tensor("wkT", [E, E], BF16, kind="ExternalInput").ap(),
        "wvT": nc.dram_tensor("wvT", [E, E], BF16, kind="ExternalInput").ap(),
        "bq": nc.dram_tensor("bq", [E], FP32, kind="ExternalInput").ap(),
        "bk": nc.dram_tensor("bk", [E], FP32, kind="ExternalInput").ap(),
        "bv": nc.dram_tensor("bv", [E], FP32, kind="ExternalInput").ap(),
        "outT": nc.dram_tensor("outT", [E, SQ], FP32, kind="ExternalOutput").ap(),
    }
    from contextlib import ExitStack
    with tile.TileContext(nc) as tc:
        with ExitStack() as ctx:
            build_kernel(ctx, tc, io)
    nc.compile()
    return nc


def make_in_maps(x, wq_w, wq_b, wk_w, wk_b, wv_w, wv_b):
    bf = ml_dtypes.bfloat16
    xT_all = np.ascontiguousarray(np.transpose(np.asarray(x, np.float32),
                                               (0, 2, 1))).astype(bf)
    wqT = np.ascontiguousarray(np.asarray(wq_w, np.float32).T).astype(bf)
    wkT = np.ascontiguousarray(np.asarray(wk_w, np.float32).T).astype(bf)
    wvT = np.ascontiguousarray(np.asarray(wv_w, np.float32).T).astype(bf)
    bq = np.asarray(wq_b, np.float32)
    bk = np.asarray(wk_b, np.float32)
    bv = np.asarray(wv_b, np.float32)
    in_maps = []
    for c in range(N_CORES):
        b, h = divmod(c, 2)
        in_maps.append({
            "xo": np.ascontiguousarray(xT_all[b][:, h * SQ:(h + 1) * SQ]),
            "wqT": wqT, "wkT": wkT, "wvT": wvT,
            "bq": bq, "bk": bk, "bv": bv,
        })
    return in_maps


def assemble_out(results):
    out = np.empty((B, S, E), np.float32)
    for c in range(N_CORES):
        b, h = divmod(c, 2)
        out[b, h * SQ:(h + 1) * SQ, :] = results[c]["outT"].T
    return out


_NC_CACHE = None


def kernel(x, wq_w, wq_b, wk_w, wk_b, wv_w, wv_b):
    global _NC_CACHE
    if _NC_CACHE is None:
        _NC_CACHE = build_program()
    in_maps = make_in_maps(x, wq_w, wq_b, wk_w, wk_b, wv_w, wv_b)
    try:
        res = run_bass_kernel_spmd(_NC_CACHE, in_maps, list(range(N_CORES)))
    except Exception:
        # transient axon/device hiccups happen; one retry
        res = run_bass_kernel_spmd(_NC_CACHE, in_maps, list(range(N_CORES)))
    return assemble_out(res.results)


# revision 3
# speedup vs baseline: 1.0182x; 1.0031x over previous
"""Single-head attention (B=4, S=2048, E=1024) on 8 TRN2 NeuronCores.

Sharding: core c -> (batch b = c//2, sequence-half h = c%2).

v5: CC warm-up dummy, split kh/vh scratch slots, earlier first slab. v3 notes: stream runs at the 216ns/MM roofline, so v3 removes
the remaining stalls found in the v2 trace:
  - wq/wv ship as slab DMAs QUEUED BEHIND wk/xo on the same channels:
    v2's batched wq/wv transfers ran concurrently and stole HBM
    bandwidth from the first-phase slabs (3.7us of et-outer stalls).
  - K gather-backs moved to the gpsimd queue: in v2 they sat AHEAD of
    bounce_v in the sync FIFO while waiting on the K collectives, and
    the first scores exp (WAR on the shared kh/vh scratch) stalled
    2.9us behind them.
  - softmax denominators: DVE running-sum over the 16 exp tiles (off
    the PE path) + ONE ones[128,128] matmul per q-slice that both
    cross-partition-reduces and broadcasts -> saves 30 of 32
    denominator matmuls (6.5us) AND kills the reciprocal DRAM
    round-trip (reciprocal reads the broadcast PSUM directly).
  - PV is qs-outer so qs=1's reciprocal has 27us of slack.
"""

import numpy as np
import ml_dtypes

import concourse.bass as bass
import concourse.tile as tile
from concourse import bacc, mybir
from concourse.bass_utils import run_bass_kernel_spmd

B, S, E = 4, 2048, 1024
N_CORES = 8
SQ = S // 2
P = 128
NT = 512
ET = E // P        # 8
KT = S // P        # 16
KTH = SQ // P      # 8 own-half k tiles
FP32 = mybir.dt.float32
FP32R = mybir.dt.float32r
BF16 = mybir.dt.bfloat16
SCALE = 1.0 / np.sqrt(E).astype(np.float32)
SHIFT = -4.0
PAIRS = [[0, 1], [2, 3], [4, 5], [6, 7]]


def build_kernel(ctx, tc, io):
    nc = tc.nc
    xo, wqT, wkT, wvT, bq, bk, bv, outT = (
        io["xo"], io["wqT"], io["wkT"], io["wvT"],
        io["bq"], io["bk"], io["bv"], io["outT"],
    )

    singles = ctx.enter_context(tc.tile_pool(name="singles", bufs=1))
    results = ctx.enter_context(tc.tile_pool(name="results", bufs=1))
    xp_pool = ctx.enter_context(tc.tile_pool(name="xp", bufs=1))
    outp = ctx.enter_context(tc.tile_pool(name="outp", bufs=3))
    dram = ctx.enter_context(tc.tile_pool(name="dram", bufs=1, space="DRAM"))
    ps_a = ctx.enter_context(tc.tile_pool(name="ps_a", bufs=4, space="PSUM"))
    ps_b = ctx.enter_context(tc.tile_pool(name="ps_b", bufs=2, space="PSUM"))
    ps_c = ctx.enter_context(tc.tile_pool(name="ps_c", bufs=2, space="PSUM"))

    # ---- ScalarE LUT warm-up: force Identity/Exp ACT_TABLE_LOADs before
    # the input DMA stream.
    warm = singles.tile([1, 4], FP32)
    warmb = singles.tile([1, 1], FP32)
    nc.vector.memset(warm, 0.0)
    nc.vector.memset(warmb, 0.0)
    nc.scalar.activation(out=warm[:, 0:2], in_=warm[:, 0:2],
                         func=mybir.ActivationFunctionType.Identity,
                         bias=warmb, scale=1.0)
    nc.scalar.activation(out=warm[:, 2:4], in_=warm[:, 2:4],
                         func=mybir.ActivationFunctionType.Exp,
                         bias=warmb, scale=1.0)

    # Tiny dummy collective fired at ~8us: absorbs the 11-18us CC
    # firmware start-delay so the real K0 gather starts promptly. Its
    # 16-byte input ships from the (already memset) warm tile as the
    # very first sync DMA.
    warm_cc_in = dram.tile([1, 1], FP32, name="warm_cc_in")
    warm_cc_out = dram.tile([2, 1], FP32, name="warm_cc_out")
    nc.sync.dma_start(out=warm_cc_in, in_=warmb[0:1, :])
    nc.gpsimd.collective_compute(
        "AllGather", mybir.AluOpType.bypass, replica_groups=PAIRS,
        ins=[warm_cc_in[:, :]], outs=[warm_cc_out[:, :]])

    # ---- input staging. Tiny bias DMAs first (they gate the first ACTs).
    bq_sb = singles.tile([P, ET], FP32)
    bk_sb = singles.tile([P, ET], FP32)
    bv_bc = singles.tile([P, E], FP32)
    nc.gpsimd.dma_start(out=bk_sb, in_=bk.rearrange("(t p) -> p t", p=P))
    nc.gpsimd.dma_start(out=bq_sb, in_=bq.rearrange("(t p) -> p t", p=P))
    nc.gpsimd.dma_start(out=bv_bc, in_=bv.partition_broadcast(P))

    # wk/xo slabs feed the et-outer first phase; wq/wv slabs go BEHIND
    # them on the same queues so their transfers can't steal HBM
    # bandwidth from the critical first 4MB.
    wk_sb = singles.tile([P, ET, E], BF16)
    wv_sb = singles.tile([P, ET, E], BF16)
    wq_sb = singles.tile([P, ET, E], BF16)
    xo_sb = singles.tile([P, ET, SQ], BF16)
    # The ks=0 et-outer phase only needs wk (2MB) + the FIRST COLUMN
    # HALF of xo (1MB): ship xo as column-halves so the critical set is
    # 3MB and slab delivery stays ahead of the warm PE (1.73us/step).
    # Everything else (xo 2nd half, wq, wv) trails on the scalar queue
    # FIFO; sync stays free so the K bounces transfer promptly.
    def chunk(q, dst_sb, src, t, n, cols):
        r = slice(t * P, (t + n) * P)
        q.dma_start(out=dst_sb[:, t:t + n, cols],
                    in_=src[r, cols].rearrange("(t p) f -> p t f", p=P))
    for (t, n) in [(0, 1), (1, 1), (2, 2), (4, 2), (6, 2)]:
        chunk(nc.sync, wk_sb, wkT, t, n, slice(None))
        chunk(nc.scalar, xo_sb, xo, t, n, slice(0, NT))
    nc.scalar.dma_start(out=xo_sb[:, :, NT:],
                        in_=xo[:, NT:].rearrange("(t p) f -> p t f", p=P))
    nc.scalar.dma_start(out=wq_sb, in_=wqT.rearrange("(t p) f -> p t f", p=P))
    nc.scalar.dma_start(out=wv_sb, in_=wvT.rearrange("(t p) f -> p t f", p=P))

    ones_b = singles.tile([P, P], BF16)
    nc.vector.memset(ones_b, 1.0)
    shift_sb = singles.tile([P, 1], FP32)
    nc.vector.memset(shift_sb, SHIFT)

    qT_sb = results.tile([P, ET, SQ], BF16)
    kT_sb = results.tile([P, ET, S], BF16)
    v_sb = results.tile([P, KT, E], BF16)
    # kh/vh scratch in two separate pool slots so the exp outputs that
    # recycle the kh slot don't WAR-wait on the (late) bounce_v read.
    kh_sb = xp_pool.tile([P, ET, SQ], BF16, tag="xpk")
    vh_sb = xp_pool.tile([P, ET, SQ], BF16, tag="xpv")

    bounce_k = [dram.tile([SQ, NT], BF16, name=f"bounce_k{i}", tag=f"bk{i}")
                for i in range(2)]
    gath_k = [dram.tile([S, NT], BF16, name=f"gath_k{i}", tag=f"gk{i}")
              for i in range(2)]
    bounce_v = dram.tile([SQ, E], BF16)
    gath_v = dram.tile([S, E], BF16)

    ident = mybir.ActivationFunctionType.Identity

    # ---- K^T own half [f, k'].
    # ks=0: contraction(et)-outer across all 8 PSUM banks -> first matmul
    # only needs slab 0 of wk/xo. ks=1: ft-outer (slabs all resident).
    ks0 = slice(0, NT)
    ps_k = ([ps_a.tile([P, NT], FP32, name=f"ps_ka{i}", tag="psa")
             for i in range(4)]
            + [ps_b.tile([P, NT], FP32, name=f"ps_kb{i}", tag="psb")
               for i in range(2)]
            + [ps_c.tile([P, NT], FP32, name=f"ps_kc{i}", tag="psc")
               for i in range(2)])
    for et in range(ET):
        for ft in range(ET):
            nc.tensor.matmul(ps_k[ft], lhsT=wk_sb[:, et, ft * P:(ft + 1) * P],
                             rhs=xo_sb[:, et, ks0],
                             start=(et == 0), stop=(et == ET - 1))
    for ft in range(ET):
        nc.scalar.activation(out=kh_sb[:, ft, ks0], in_=ps_k[ft], func=ident,
                             bias=bk_sb[:, ft:ft + 1], scale=1.0)
    nc.sync.dma_start(out=bounce_k[0].rearrange("(t p) n -> p t n", p=P),
                      in_=kh_sb[:, :, ks0])
    nc.gpsimd.collective_compute(
        "AllGather", mybir.AluOpType.bypass, replica_groups=PAIRS,
        ins=[bounce_k[0][:, :]], outs=[gath_k[0][:, :]])

    ks1 = slice(NT, SQ)
    for ft in range(ET):
        fr = slice(ft * P, (ft + 1) * P)
        ps = ps_a.tile([P, NT], FP32, tag="psa")
        for et in range(ET):
            nc.tensor.matmul(ps, lhsT=wk_sb[:, et, fr], rhs=xo_sb[:, et, ks1],
                             start=(et == 0), stop=(et == ET - 1))
        nc.scalar.activation(out=kh_sb[:, ft, ks1], in_=ps, func=ident,
                             bias=bk_sb[:, ft:ft + 1], scale=1.0)
    nc.sync.dma_start(out=bounce_k[1].rearrange("(t p) n -> p t n", p=P),
                      in_=kh_sb[:, :, ks1])
    nc.gpsimd.collective_compute(
        "AllGather", mybir.AluOpType.bypass, replica_groups=PAIRS,
        ins=[bounce_k[1][:, :]], outs=[gath_k[1][:, :]])

    # ---- V own half [k', f] -> bounce (sync) -> gather (gpsimd)
    for kt in range(KTH):
        kr = slice(kt * P, (kt + 1) * P)
        for fs in range(E // NT):
            fr = slice(fs * NT, (fs + 1) * NT)
            ps = ps_a.tile([P, NT], FP32, tag="psa")
            for et in range(ET):
                nc.tensor.matmul(ps, lhsT=xo_sb[:, et, kr], rhs=wv_sb[:, et, fr],
                                 start=(et == 0), stop=(et == ET - 1))
            nc.vector.tensor_add(vh_sb[:, kt, fr], ps, bv_bc[:, fr])
    nc.sync.dma_start(out=bounce_v.rearrange("(t p) f -> p t f", p=P),
                      in_=vh_sb[:, :, :])
    nc.gpsimd.collective_compute(
        "AllGather", mybir.AluOpType.bypass, replica_groups=PAIRS,
        ins=[bounce_v[:, :]], outs=[gath_v[:, :]])

    # K gather-backs on gpsimd, AFTER the V collective trigger in queue
    # order (they wait on the K collectives anyway; keeping them off the
    # sync queue unblocks bounce_v).
    for ks in range(2):
        for hh in range(2):
            lo = hh * SQ + ks * NT
            nc.gpsimd.dma_start(
                out=kT_sb[:, :, lo:lo + NT],
                in_=gath_k[ks][hh * SQ:(hh + 1) * SQ, :]
                .rearrange("(t p) n -> p t n", p=P))
    # V gather-back on sync (idle after bounce_v), in halves so the
    # first PV chains start as soon as kt 0-7 land.
    for vh2 in range(2):
        lo = vh2 * KTH
        nc.sync.dma_start(
            out=v_sb[:, lo:lo + KTH, :],
            in_=gath_v[lo * P:(lo + KTH) * P, :]
            .rearrange("(t p) f -> p t f", p=P))

    # ---- Q^T = Wq x_own + bq   [f, q]  (overlaps the gathers)
    for qs in range(SQ // NT):
        qr = slice(qs * NT, (qs + 1) * NT)
        for ft in range(ET):
            fr = slice(ft * P, (ft + 1) * P)
            ps = ps_a.tile([P, NT], FP32, tag="psa")
            for et in range(ET):
                nc.tensor.matmul(ps, lhsT=wq_sb[:, et, fr], rhs=xo_sb[:, et, qr],
                                 start=(et == 0), stop=(et == ET - 1))
            nc.scalar.activation(out=qT_sb[:, ft, qr], in_=ps, func=ident,
                                 bias=bq_sb[:, ft:ft + 1], scale=1.0)

    # ---- scores^T and P = exp(S^T * scale + shift)   [k, q]
    # Denominator: DVE running sum over the 16 exp tiles (acc, fp32r so
    # it can feed a matmul directly), then ONE ones[128,128] matmul that
    # cross-partition-reduces AND broadcasts the result to all
    # partitions; reciprocal reads that PSUM directly.  The denominator
    # matmul for qs is issued a few chains into the NEXT phase so the PE
    # never waits on the DVE accumulation.
    p_k = xp_pool.tile([P, ET, SQ], BF16, tag="xpk")
    p_v = xp_pool.tile([P, ET, SQ], BF16, tag="xpv")

    def p_tile(kt, qr):
        return p_k[:, kt, qr] if kt < ET else p_v[:, kt - ET, qr]
    acc = singles.tile([P, 2, NT], FP32, name="acc_den")
    acc_b = singles.tile([P, 2, NT], BF16, name="acc_den_b")
    recip_bc = singles.tile([P, SQ], FP32)
    ps_den = [None, None]

    # kt order visits the ks=0-gathered blocks first: the ks=1 gather-
    # backs land ~15us later (CC stream is serialized) and softmax is
    # k-order-invariant, so consume what's ready first.
    KT_ORDER = [0, 1, 2, 3, 8, 9, 10, 11, 4, 5, 6, 7, 12, 13, 14, 15]

    def scores_phase(qs):
        qr = slice(qs * NT, (qs + 1) * NT)
        for ki, kt in enumerate(KT_ORDER):
            kr = slice(kt * P, (kt + 1) * P)
            ps = ps_a.tile([P, NT], FP32, tag="psa")
            for et in range(ET):
                nc.tensor.matmul(ps, lhsT=kT_sb[:, et, kr], rhs=qT_sb[:, et, qr],
                                 start=(et == 0), stop=(et == ET - 1))
            nc.scalar.activation(out=p_tile(kt, qr), in_=ps,
                                 func=mybir.ActivationFunctionType.Exp,
                                 bias=shift_sb[:, 0:1], scale=float(SCALE))
            # DVE running sum (off the PE path)
            if ki == 1:
                nc.vector.tensor_add(acc[:, qs, :], p_tile(KT_ORDER[0], qr),
                                     p_tile(kt, qr))
            elif ki > 1:
                nc.vector.tensor_add(acc[:, qs, :], acc[:, qs, :],
                                     p_tile(kt, qr))
                if ki == KT - 1:
                    nc.vector.tensor_copy(out=acc_b[:, qs, :],
                                          in_=acc[:, qs, :])
            if qs == 1 and ki == 3:
                denominator_matmul(0)

    def denominator_matmul(qs):
        qr = slice(qs * NT, (qs + 1) * NT)
        ps_den[qs] = ps_b.tile([P, NT], FP32, tag="psb", name=f"ps_den{qs}")
        nc.tensor.matmul(ps_den[qs], lhsT=ones_b, rhs=acc_b[:, qs, :],
                         start=True, stop=True)
        nc.vector.reciprocal(out=recip_bc[:, qr], in_=ps_den[qs])

    scores_phase(0)
    scores_phase(1)

    # ---- O^T = V^T P, normalize, out (qs-outer: qs=1's reciprocal has
    # a whole qs=0 sweep of slack)
    for qs in range(SQ // NT):
        for ft in range(ET):
            fr = slice(ft * P, (ft + 1) * P)
            last = (qs == SQ // NT - 1) and (ft == ET - 1)
            # the very last tile runs as two half-width chains so its
            # evacuation overlaps the tail instead of serializing it
            for qq in ([slice(qs * NT, (qs + 1) * NT)] if not last else
                       [slice(qs * NT + h * (NT // 2),
                              qs * NT + (h + 1) * (NT // 2)) for h in range(2)]):
                ps = ps_c.tile([P, qq.stop - qq.start], FP32, tag="psc")
                for kt in range(KT):
                    nc.tensor.matmul(ps, lhsT=v_sb[:, kt, fr],
                                     rhs=p_tile(kt, qq),
                                     start=(kt == 0), stop=(kt == KT - 1))
                if qs == 0 and ft == 0:
                    denominator_matmul(1)
                ot = outp.tile([P, qq.stop - qq.start], FP32)
                nc.vector.tensor_mul(ot, ps, recip_bc[:, qq])
                nc.scalar.dma_start(out=outT[fr, qq], in_=ot)


def build_program():
    nc = bacc.Bacc("TRN2", target_bir_lowering=False, debug=False,
                   num_devices=N_CORES)
    io = {
        "xo": nc.dram_tensor("xo", [E, SQ], BF16, kind="ExternalInput").ap(),
        "wqT": nc.dram_tensor("wqT", [E, E], BF16, kind="ExternalInput").ap(),
        "wkT": nc.dram_tensor("wkT", [E, E], BF16, kind="ExternalInput").ap(),
        "wvT": nc.dram_tensor("wvT", [E, E], BF16, kind="ExternalInput").ap(),
        "bq": nc.dram_tensor("bq", [E], FP32, kind="ExternalInput").ap(),
        "bk": nc.dram_tensor("bk", [E], FP32, kind="ExternalInput").ap(),
        "bv": nc.dram_tensor("bv", [E], FP32, kind="ExternalInput").ap(),
        "outT": nc.dram_tensor("outT", [E, SQ], FP32, kind="ExternalOutput").ap(),
    }
    from contextlib import ExitStack
    with tile.TileContext(nc) as tc:
        with ExitStack() as ctx:
            build_kernel(ctx, tc, io)
    nc.compile()
    return nc


def make_in_maps(x, wq_w, wq_b, wk_w, wk_b, wv_w, wv_b):
    bf = ml_dtypes.bfloat16
    xT_all = np.ascontiguousarray(np.transpose(np.asarray(x, np.float32),
                                               (0, 2, 1))).astype(bf)
    wqT = np.ascontiguousarray(np.asarray(wq_w, np.float32).T).astype(bf)
    wkT = np.ascontiguousarray(np.asarray(wk_w, np.float32).T).astype(bf)
    wvT = np.ascontiguousarray(np.asarray(wv_w, np.float32).T).astype(bf)
    bq = np.asarray(wq_b, np.float32)
    bk = np.asarray(wk_b, np.float32)
    bv = np.asarray(wv_b, np.float32)
    in_maps = []
    for c in range(N_CORES):
        b, h = divmod(c, 2)
        in_maps.append({
            "xo": np.ascontiguousarray(xT_all[b][:, h * SQ:(h + 1) * SQ]),
            "wqT": wqT, "wkT": wkT, "wvT": wvT,
            "bq": bq, "bk": bk, "bv": bv,
        })
    return in_maps


def assemble_out(results):
    out = np.empty((B, S, E), np.float32)
    for c in range(N_CORES):
        b, h = divmod(c, 2)
        out[b, h * SQ:(h + 1) * SQ, :] = results[c]["outT"].T
    return out


_NC_CACHE = None


def kernel(x, wq_w, wq_b, wk_w, wk_b, wv_w, wv_b):
    global _NC_CACHE
    if _NC_CACHE is None:
        _NC_CACHE = build_program()
    in_maps = make_in_maps(x, wq_w, wq_b, wk_w, wk_b, wv_w, wv_b)
    try:
        res = run_bass_kernel_spmd(_NC_CACHE, in_maps, list(range(N_CORES)))
    except Exception:
        # transient axon/device hiccups happen; one retry
        res = run_bass_kernel_spmd(_NC_CACHE, in_maps, list(range(N_CORES)))
    return assemble_out(res.results)
